# revision 1
# baseline (speedup 1.0000x reference)
"""Sliding-window attention (window=256) on 8 TRN2 NeuronCores.

Layout/algorithm notes
----------------------
Shapes: q,k,v [4,16,4096,64] fp32; B*H=64 (b,h) pairs sharded 8 per core
(fully local along sequence, no communication).

Per (b,h) and per 512-query block t (8 blocks per head):
  keys needed: [512t-256, 512t+512) = 6 key-chunks of 128 (global chunk
  index g = 4t-2+c, c=0..5; chunks with g<0 are skipped).
  S^T chunk  = matmul(lhsT=K^T[:,128g:128g+128] (fp32r [64,128]),
                      rhs=Q^T[:, 512t+qw_c]     (fp32r [64,|qw_c|]))
               -> PSUM [128,|qw_c|]  (scores transposed: [key, query]);
  qw_c is the chunk's valid query subrange (width 128..384); chunk pairs
  {c0,c2},{c1,c4},{c3,c5} share one PSUM bank each.
  P^T chunk  = exp(S^T * D^-1/2) via ACT (PSUM->SBUF, output rounded to
  fp32r), then out-of-band entries are zeroed by a {1,0} band-mask
  multiply split across DVE and GpSimd.  P^T slots are padded/zeroed to
  256-aligned windows so PV matmuls can share identical APs.
  O^T += matmul(lhsT=[V|1]-chunk (fp32r [128,65]), rhs=P^T slice) into
  one PSUM bank per 256-query column group; row 64 accumulates the
  softmax denominator (ones-column trick).
  Epilogue: copy O^T groups to SBUF, 4 PE transposes -> [128,65],
  reciprocal of the denominator column, per-partition scalar multiply,
  one DMA per block.

The emission order is software-pipelined (QK/exp/mask of block t is
emitted before PV/epilogue of block t-1) so the tensor engine always has
independent matmuls in flight while ACT/DVE/GpSimd work on the current
block.  Q^T/K^T are produced on-chip by PE transposes (fp32r, 4 per
PSUM bank + one wide copy).  fp32r matmuls measure ~1.7e-4 relative
error (TF32-like); the ACT exp table adds ~9e-6.
"""

import numpy as np

import concourse.bass as bass
import concourse.mybir as mybir
from concourse import bacc
from concourse.tile import TileContext
from concourse import bass_utils
from concourse.masks import make_identity

dt = mybir.dt

B, H, S, D = 4, 16, 4096, 64
W = 256                      # sliding window
N_CORES = 8
BH = (B * H) // N_CORES      # (b,h) pairs per core = 8
QT = 512                     # queries per block
NB = S // QT                 # blocks per (b,h) = 8
NT = S // 128                # 128-tiles per (b,h) = 32
SCALE = float(D) ** -0.5

# per-chunk query windows (relative to block start), c = 0..5
QW = [(max(0, 128 * (c - 2)), min(QT, 128 * (c - 2) + 384)) for c in range(6)]
# P^T slot windows, padded to 256-aligned PV column groups; the pad region
# (slot minus QW) is zero-filled once so PV matmuls share identical group APs
SLOT = [(0, 256), (0, 256), (0, 512), (0, 512), (256, 512), (256, 512)]
# PV column groups (2 per block) and their member chunks
PV_GROUPS = [(0, [0, 1, 2, 3]), (256, [2, 3, 4, 5])]
SLOT_BASE = {}
_off = 0
for _c in range(6):
    SLOT_BASE[_c] = _off - SLOT[_c][0]
    _off += SLOT[_c][1] - SLOT[_c][0]
PT_W = _off
# pad regions (cols) that must stay zero: slot minus computed window
PT_PADS = []
for _c in range(6):
    for _p0, _p1 in [(SLOT[_c][0], QW[_c][0]), (QW[_c][1], SLOT[_c][1])]:
        if _p1 > _p0:
            PT_PADS.append((SLOT_BASE[_c] + _p0, SLOT_BASE[_c] + _p1))

# S^T chunk pairs sharing one PSUM bank (widths sum <= 512 fp32)
ST_BANK = {0: (0, 0), 2: (0, 128), 1: (1, 0), 4: (1, 256), 3: (2, 0), 5: (2, 384)}


def _band_valid_np():
    kl = np.arange(128)[:, None]
    m = np.arange(384)[None, :]
    return (m - 256 <= kl) & (kl <= m)


def _band_mask_np():
    """band[kl, m]: multiplicative mask, 1 where valid (m-256 <= kl <= m)."""
    return np.where(_band_valid_np(), np.float32(1.0), np.float32(0.0))


def _chunk_mask_ops(c):
    """For chunk c, list of (local j0, mega-mask col offset) for 128-wide
    subranges of qw_c that are not entirely valid."""
    j0s = []
    q0, q1 = QW[c]
    off = 128 * max(0, 2 - c)
    valid = _band_valid_np()
    for j0 in range(0, q1 - q0, 128):
        m0 = j0 + off
        if not valid[:, m0:m0 + 128].all():
            j0s.append((j0, m0))
    return j0s


MASK_OPS = {c: _chunk_mask_ops(c) for c in range(6)}


def build_core_kernel(n_bh=BH):
    nc = bacc.Bacc("TRN2", target_bir_lowering=False)
    qd = nc.dram_tensor("q", [n_bh * S, D], dt.float32, kind="ExternalInput")
    kd = nc.dram_tensor("k", [n_bh * S, D], dt.float32, kind="ExternalInput")
    vd = nc.dram_tensor("v", [n_bh * S, D], dt.float32, kind="ExternalInput")
    md = nc.dram_tensor("band_mask", [128, 384], dt.float32, kind="ExternalInput")
    od = nc.dram_tensor("o", [n_bh * S, D], dt.float32, kind="ExternalOutput")

    with TileContext(nc) as tc:
        with (
            tc.tile_pool(name="const", bufs=1) as constp,
            tc.tile_pool(name="bigio", bufs=2) as bigio,
            tc.tile_pool(name="work", bufs=2) as work,
            tc.tile_pool(name="pst", bufs=3, space="PSUM") as pst,     # prep transposes
            tc.tile_pool(name="psst", bufs=1, space="PSUM") as psst,   # S^T pair-banks (3 tags)
            tc.tile_pool(name="psot", bufs=1, space="PSUM") as psot,   # O^T accum + epi transposes
        ):
            ident = constp.tile([128, 128], dt.float32)
            make_identity(nc, ident)
            ident_r = constp.tile([128, 128], dt.float32r)
            nc.vector.tensor_copy(ident_r[:], ident[:])
            mega = constp.tile([128, 384], dt.float32r)
            nc.gpsimd.dma_start(mega[:], md[:])   # cast {1,0} fp32 -> fp32r
            ones32 = constp.tile([128, NT], dt.float32)
            nc.vector.memset(ones32[:], 1.0)
            zeros32 = constp.tile([128, 128], dt.float32)
            nc.vector.memset(zeros32[:], 0.0)

            # persistent, manually double-buffered P^T tiles; pad columns are
            # zeroed once here and never written again
            pts = [constp.tile([128, PT_W], dt.float32r, name=f"ptbuf{i}")
                   for i in range(3)]
            for ptb in pts:
                for p0, p1 in PT_PADS:
                    for x0 in range(p0, p1, 128):
                        x1 = min(x0 + 128, p1)
                        nc.vector.tensor_copy(ptb[:, x0:x1], zeros32[:, 0:x1 - x0])

            for bh in range(n_bh):
                base = bh * S
                # ---- load natural-layout q/k/v, SWDGE-cast to fp32r
                qnat = bigio.tile([128, NT * D], dt.float32r, tag="qnat", name="qnat")
                knat = bigio.tile([128, NT * D], dt.float32r, tag="knat", name="knat")
                qsl = qd[base:base + S, :].rearrange("(t p) d -> p t d", p=128)
                ksl = kd[base:base + S, :].rearrange("(t p) d -> p t d", p=128)
                nc.gpsimd.dma_start(qnat[:].rearrange("p (t d) -> p t d", d=D), qsl)
                nc.gpsimd.dma_start(knat[:].rearrange("p (t d) -> p t d", d=D), ksl)

                vt = bigio.tile([128, NT * (D + 1)], dt.float32r, tag="vt", name="vt")
                vt3 = vt[:].rearrange("p (g e) -> p g e", e=D + 1)
                vsl = vd[base:base + S, :].rearrange("(g p) d -> p g d", p=128)
                nc.gpsimd.dma_start(vt3[:, :, 0:D], vsl)     # SWDGE casts fp32->fp32r
                # ones column: cast fp32 1.0 -> proper fp32r bits via DVE copy
                nc.vector.tensor_copy(vt3[:, :, D], ones32[:])

                # ---- transpose to Q^T/K^T [64, S] fp32r
                # 4 PE transposes into one PSUM bank, then a single wide copy
                qt = bigio.tile([64, S], dt.float32r, tag="qt", name="qt")
                kt = bigio.tile([64, S], dt.float32r, tag="kt", name="kt")
                for half, (nat, tr) in enumerate([(qnat, qt), (knat, kt)]):
                    for i0 in range(0, NT, 4):
                        ptr = pst.tile([64, 512], dt.float32r, tag="trp", name="ptr")
                        for u in range(4):
                            i = i0 + u
                            nc.tensor.transpose(
                                ptr[:, 128 * u:128 * (u + 1)],
                                nat[:, D * i:D * (i + 1)], ident_r[:])
                        if (i0 // 4 + half) % 2 == 0:
                            nc.scalar.copy(tr[:, 128 * i0:128 * (i0 + 4)], ptr[:])
                        else:
                            nc.vector.tensor_copy(
                                tr[:, 128 * i0:128 * (i0 + 4)], ptr[:])

                # ---- blocks
                def emit_qk(t):
                    """QK chunk matmuls + exp + masks for block t."""
                    chunks = [c for c in range(6) if 4 * t - 2 + c >= 0]
                    pt = pts[(bh * NB + t) % 3]
                    stt = {}
                    for c in chunks:
                        g = 4 * t - 2 + c
                        q0, q1 = QW[c]
                        w = q1 - q0
                        bank, boff = ST_BANK[c]
                        if bank not in stt:
                            stt[bank] = psst.tile(
                                [128, 512], dt.float32, tag=f"st{bank}",
                                name=f"st{bank}")
                        st = stt[bank]
                        nc.tensor.matmul(
                            st[:, boff:boff + w],
                            kt[:, 128 * g:128 * (g + 1)],
                            qt[:, QT * t + q0:QT * t + q1],
                            start=True, stop=True,
                        )
                        # exp(S^T * scale) -> P^T slot (fp32r), then zero the
                        # out-of-band entries with a {1,0} multiply (DVE/Pool)
                        po = SLOT_BASE[c] + q0
                        nc.scalar.activation(
                            pt[:, po:po + w], st[:, boff:boff + w],
                            mybir.ActivationFunctionType.Exp, scale=SCALE,
                        )
                        for mi_, (j0, m0) in enumerate(MASK_OPS[c]):
                            eng = nc.vector if (c + mi_) % 2 == 0 else nc.gpsimd
                            eng.tensor_tensor(
                                pt[:, po + j0:po + j0 + 128],
                                pt[:, po + j0:po + j0 + 128],
                                mega[:, m0:m0 + 128],
                                op=mybir.AluOpType.mult,
                            )

                def emit_pv(t):
                    """PV accumulation + normalize + transpose + store for t."""
                    chunks = [c for c in range(6) if 4 * t - 2 + c >= 0]
                    pt = pts[(bh * NB + t) % 3]
                    osb = work.tile([65, QT], dt.float32, tag="osb", name="osb")
                    for j, (col0, group) in enumerate(PV_GROUPS):
                        members = [c for c in group if c in chunks]
                        otj = psot.tile([65, 256], dt.float32, tag=f"ot{j}",
                                        name=f"ot{j}")
                        for mi, c in enumerate(members):
                            g = 4 * t - 2 + c
                            po = SLOT_BASE[c] + col0
                            nc.tensor.matmul(
                                otj[:], vt3[:, g, :], pt[:, po:po + 256],
                                start=(mi == 0), stop=(mi == len(members) - 1),
                            )
                        # DVE, not ACT: keeps the scalar engine free for exps
                        nc.vector.tensor_copy(osb[:, col0:col0 + 256], otj[:])

                    otr = psot.tile([128, 4 * 65], dt.float32, tag="ot0", name="otr")
                    for j in range(4):
                        nc.tensor.transpose(
                            otr[:, 65 * j:65 * (j + 1)],
                            osb[:, 128 * j:128 * (j + 1)], ident[0:65, 0:65],
                        )
                    rc = work.tile([128, 4], dt.float32, tag="rc", name="rc")
                    otr3 = otr[:].rearrange("p (j e) -> p j e", e=65)
                    nc.vector.reciprocal(rc[:], otr3[:, :, 64])
                    outsb = work.tile([128, 4 * D], dt.float32, tag="outsb",
                                      name="outsb")
                    for j in range(4):
                        nc.vector.tensor_scalar_mul(
                            outsb[:, D * j:D * (j + 1)],
                            otr[:, 65 * j:65 * j + 64],
                            rc[:, j:j + 1],
                        )
                    osl = od[base + QT * t:base + QT * (t + 1), :].rearrange(
                        "(j p) d -> p j d", p=128)
                    nc.sync.dma_start(
                        osl, outsb[:].rearrange("p (j d) -> p j d", d=D))

                # software pipeline: emit QK(t) ahead of PV(t-1) so the PE
                # always has independent matmul work while exp/masks of the
                # current block complete on ACT/DVE/Pool
                for t in range(NB):
                    emit_qk(t)
                    if t > 0:
                        emit_pv(t - 1)
                emit_pv(NB - 1)

    nc.finalize()
    return nc


_NC_CACHE = []


def _get_nc():
    if not _NC_CACHE:
        _NC_CACHE.append(build_core_kernel())
    return _NC_CACHE[0]


def make_in_maps(q, k, v):
    qr = np.ascontiguousarray(np.asarray(q, dtype=np.float32).reshape(B * H, S, D))
    kr = np.ascontiguousarray(np.asarray(k, dtype=np.float32).reshape(B * H, S, D))
    vr = np.ascontiguousarray(np.asarray(v, dtype=np.float32).reshape(B * H, S, D))
    band = np.ascontiguousarray(_band_mask_np())

    in_maps = []
    for i in range(N_CORES):
        in_maps.append({
            "q": np.ascontiguousarray(qr[BH * i:BH * (i + 1)].reshape(BH * S, D)),
            "k": np.ascontiguousarray(kr[BH * i:BH * (i + 1)].reshape(BH * S, D)),
            "v": np.ascontiguousarray(vr[BH * i:BH * (i + 1)].reshape(BH * S, D)),
            "band_mask": band,
        })
    return in_maps


def gather_out(res):
    out = np.empty((B * H, S, D), dtype=np.float32)
    for i in range(N_CORES):
        out[BH * i:BH * (i + 1)] = res.results[i]["o"].reshape(BH, S, D)
    return out.reshape(B, H, S, D)


def kernel(q, k, v):
    nc = _get_nc()
    in_maps = make_in_maps(q, k, v)
    res = bass_utils.run_bass_kernel_spmd(nc, in_maps, core_ids=list(range(N_CORES)))
    return gather_out(res)



# revision 2
# speedup vs baseline: 1.0184x; 1.0184x over previous
"""Sliding-window attention (window=256) on 8 TRN2 NeuronCores — v2.

Design (vs the fp32r baseline):
- All compute in bf16 (inputs SWDGE-cast on load): every matmul streams at
  1 cycle/row regardless of width, weight loads get FWL.
- Heads processed in PAIRS: head A's transposed Q^T/K^T live on partitions
  0-63, head B's on 64-127 (produced by fused [128,128] PE transposes of
  column-interleaved natural tiles).  The two heads' QK chunk matmuls use
  row groups (0,0)/(64,0) via base-partition-64 APs and run CONCURRENTLY
  on the PE array (contraction is only d=64).
- Transposes are regular matmuls against a bf16 identity (counts as PE-busy
  for the HAM clock gate, unlike transpose-mode), emitted interleaved with
  block compute so the PE never idles into a re-throttle window.
- Scores go to one shared [128,2048] fp32 PSUM tile (4 banks) in 2 waves:
  wave1 = chunks {c0,c2,c1,c4} of both heads, wave2 = {c3,c5} (reuses banks
  after the wave-1 exp drains).  ONE merged exp per wave on ACT
  ([128,2048] then [128,1024]) writing bf16 P^T.
- Band masking = 2 wide bf16 tensor_tensor multiplies against a precomputed
  {1,0} mega-mask (DVE 4x mode: all-SBUF, all-2-byte).
- PV is per-chunk into a [65,512] fp32 PSUM bank per head using has_written
  semantics: the c2 matmul (cols 0:384) is the only start=True (clears the
  bank's has_written bits); every other chunk accumulates where bits are set
  and overwrites where they aren't.  V carries a ones column so row 64
  accumulates the softmax denominator.
- Epilogue: O^T [65,512] is cast-copied to SBUF bf16 [80,512], transposed by
  the DMA XBAR (nc.sync dma transpose) to [128,4,80] (query q = 4p+j), then
  reciprocal + one broadcast multiply (all SBUF) and a contiguous store.
"""

import numpy as np

import concourse.bass as bass
import concourse.mybir as mybir
from concourse import bacc
from concourse.tile import TileContext
from concourse import bass_utils
from concourse.masks import make_identity

dt = mybir.dt

B, H, S, D = 4, 16, 4096, 64
W = 256
N_CORES = 8
BH = (B * H) // N_CORES      # heads per core = 8
NPAIR = BH // 2              # head pairs per core = 4
QT = 512                     # queries per block
NB = S // QT                 # blocks per head = 8
NT = S // 128                # 128-tiles per head = 32
SCALE = float(D) ** -0.5

# chunk c covers key chunk g = 4t-2+c; query window within the block:
QW = {0: (0, 128), 1: (0, 256), 2: (0, 384),
      3: (128, 512), 4: (256, 512), 5: (384, 512)}
W1 = [0, 2, 1, 4]            # wave-1 chunks (banks 0-1 per head)
W2 = [3, 5]                  # wave-2 chunks
ST1_OFF = {0: 0, 2: 128, 1: 512, 4: 768}
ST2_OFF = {3: 0, 5: 384}


def st_off(h, c):
    """Column offset of (head h, chunk c) scores within its wave's
    [128,1024] ST region (wave1 regions are per-head, wave2 is shared)."""
    if c in ST1_OFF:
        return ST1_OFF[c]
    return 512 * h + ST2_OFF[c]


def pt_off(h, c):
    """Column offset of (head h, chunk c) probabilities in the [128,3072] P^T."""
    if c in ST1_OFF:
        return 1024 * h + ST1_OFF[c]
    return 2048 + 512 * h + ST2_OFF[c]


def build_mega_mask():
    """{1,0} float32 [128, 3072] band mask laid out to match pt_off."""
    m = np.zeros((128, 3072), dtype=np.float32)
    kl = np.arange(128)[:, None]
    for h in (0, 1):
        for c in range(6):
            q0, q1 = QW[c]
            qi = np.arange(q0, q1)[None, :]
            key = 128 * (c - 2) + kl
            valid = (key >= qi - 256) & (key <= qi)
            off = pt_off(h, c)
            m[:, off:off + (q1 - q0)] = valid.astype(np.float32)
    return m


def build_core_kernel(n_bh=BH):
    nc = bacc.Bacc("TRN2", target_bir_lowering=False)
    qd = nc.dram_tensor("q", [n_bh * S, D], dt.float32, kind="ExternalInput")
    kd = nc.dram_tensor("k", [n_bh * S, D], dt.float32, kind="ExternalInput")
    vd = nc.dram_tensor("v", [n_bh * S, D], dt.float32, kind="ExternalInput")
    md = nc.dram_tensor("band_mask", [128, 3072], dt.float32,
                        kind="ExternalInput")
    od = nc.dram_tensor("o", [n_bh * S, D], dt.float32, kind="ExternalOutput")

    with TileContext(nc) as tc:
        with (
            tc.tile_pool(name="const", bufs=1) as constp,
            tc.tile_pool(name="bigio", bufs=2) as bigio,
            tc.tile_pool(name="qkt", bufs=2) as qktp,
            tc.tile_pool(name="ptp", bufs=3) as ptp,
            tc.tile_pool(name="work", bufs=2) as work,
            tc.tile_pool(name="psst", bufs=3, space="PSUM") as psst,
            tc.tile_pool(name="psot", bufs=1, space="PSUM") as psot,
        ):
            ident = constp.tile([128, 128], dt.float32)
            make_identity(nc, ident)
            identb = constp.tile([128, 128], dt.bfloat16)
            nc.vector.tensor_copy(identb[:], ident[:])
            mega = constp.tile([128, 3072], dt.bfloat16)
            nc.gpsimd.dma_start(mega[:], md[:])  # cast fp32 -> bf16

            def emit_loads(p):
                """SWDGE cast-loads for pair p: q/k interleaved by head within
                each 128-tile's columns, v per (chunk, head)."""
                base = 2 * p * S
                qnat = bigio.tile([128, NT * 128], dt.bfloat16, tag="qnat",
                                  name=f"qnat{p}")
                knat = bigio.tile([128, NT * 128], dt.bfloat16, tag="knat",
                                  name=f"knat{p}")
                for dn, nat in ((qd, qnat), (kd, knat)):
                    dst4 = nat[:].rearrange("p (t h d) -> p t h d", h=2, d=64)
                    for h in (0, 1):
                        src = dn[base + h * S:base + (h + 1) * S, :].rearrange(
                            "(t p) d -> p t d", p=128)
                        nc.gpsimd.dma_start(dst4[:, :, h, :], src)
                vtp = bigio.tile([128, NT * 2 * 65], dt.bfloat16, tag="vtp",
                                 name=f"vtp{p}")
                vt4 = vtp[:].rearrange("p (g h e) -> p g h e", h=2, e=65)
                for h in (0, 1):
                    vsrc = vd[base + h * S:base + (h + 1) * S, :].rearrange(
                        "(g p) d -> p g d", p=128)
                    nc.gpsimd.dma_start(vt4[:, :, h, 0:D], vsrc)
                nc.vector.memset(vt4[:, :, :, D], 1.0)
                return qnat, knat, vtp

            def alloc_qkt(p):
                qt2 = qktp.tile([128, S], dt.bfloat16, tag="qt2",
                                name=f"qt2_{p}")
                kt2 = qktp.tile([128, S], dt.bfloat16, tag="kt2",
                                name=f"kt2_{p}")
                return qt2, kt2

            def emit_prep(nats, qk2, t):
                """Transpose tiles 4t..4t+3 of q and k for the NEXT pair.
                Transpose-mode with bf16 PSUM output so the copy-out runs at
                the DVE 2-byte rate; PSUM comes from the st pool rotation
                (2KB per tile = same bank footprint as a score tile)."""
                qnat, knat, _ = nats
                qt2, kt2 = qk2
                for nat, dst in ((qnat, qt2), (knat, kt2)):
                    tr = psst.tile([128, 2048], dt.bfloat16, tag="st",
                                   name="tr")
                    for u in range(4):
                        i = 4 * t + u
                        nc.tensor.transpose(
                            tr[:, 128 * u:128 * (u + 1)],
                            nat[:, 128 * i:128 * (i + 1)], identb[:])
                    nc.vector.tensor_copy(dst[:, 512 * t:512 * (t + 1)],
                                          tr[:, 0:512])

            def emit_qk(qk2, st, t, chunks, heads):
                qt2, kt2 = qk2
                for c in chunks:
                    g = 4 * t - 2 + c
                    if g < 0:
                        continue
                    q0, q1 = QW[c]
                    for h in heads:
                        b = 64 * h
                        nc.tensor.matmul(
                            st[:, st_off(h, c):st_off(h, c) + (q1 - q0)],
                            kt2[b:b + 64, 128 * g:128 * (g + 1)],
                            qt2[b:b + 64, QT * t + q0:QT * t + q1],
                            start=True, stop=True)

            def emit_pv(vtp, pt, t):
                vt4 = vtp[:].rearrange("p (g h e) -> p g h e", h=2, e=65)
                chunks = [c for c in range(6) if 4 * t - 2 + c >= 0]
                order = [2] + [c for c in chunks if c != 2]
                ot2 = psot.tile([65, 2 * QT], dt.float32, tag="ot2",
                                name="ot2")
                for idx, c in enumerate(order):
                    g = 4 * t - 2 + c
                    q0, q1 = QW[c]
                    for h in (0, 1):
                        nc.tensor.matmul(
                            ot2[:, QT * h + q0:QT * h + q1],
                            vt4[:, g, h, :],
                            pt[:, pt_off(h, c):pt_off(h, c) + (q1 - q0)],
                            start=(idx == 0), stop=(idx == len(order) - 1),
                            skip_group_check=True)
                return ot2

            def emit_epi_head(ot2):
                """osb cast + dma transpose for both heads (merged)."""
                osb = work.tile([80, 2 * QT], dt.bfloat16, tag="osb",
                                name="osb")
                nc.vector.tensor_copy(osb[0:65, :], ot2[:])
                osbT = work.tile([128, 8 * 80], dt.bfloat16,
                                 tag="osbT", name="osbT")
                o3 = osbT[:].rearrange("p (j e) -> p j e", e=80)
                nc.sync.dma_start(o3, osb[:], transpose=True)
                return osbT

            def emit_epi_tail(osbT, p, t):
                o3 = osbT[:].rearrange("p (j e) -> p j e", e=80)
                rc = work.tile([128, 8], dt.float32, tag="rc", name="rc")
                nc.vector.reciprocal(rc[:], o3[:, :, D])
                outsb = work.tile([128, 512], dt.float32, tag="outsb",
                                  name="outsb")
                u3 = outsb[:].rearrange("p (j e) -> p j e", e=64)
                rcb = rc[:].rearrange("p (j o) -> p j o", o=1)
                nc.vector.tensor_tensor(
                    u3, o3[:, :, 0:D], rcb.broadcast_to([128, 8, 64]),
                    op=mybir.AluOpType.mult)
                for h in (0, 1):
                    base = (2 * p + h) * S
                    # dma transpose is j-major: query q = 128j + p
                    dst = od[base + QT * t:base + QT * (t + 1), :].rearrange(
                        "(j p) d -> p j d", p=128)
                    nc.sync.dma_start(dst, u3[:, 4 * h:4 * h + 4, :])

            # ---- prologue: loads for pairs 0/1; pair-0 transposes are
            # emitted inline at the head of each of its blocks
            nats = [emit_loads(0), emit_loads(1)]
            qk2s = [alloc_qkt(0)]

            for p in range(NPAIR):
                if p + 2 < NPAIR:
                    nats.append(emit_loads(p + 2))
                if p + 1 < NPAIR:
                    qk2s.append(alloc_qkt(p + 1))
                vtp_p = nats[p][2]
                for t in range(NB):
                    if p == 0:
                        emit_prep(nats[0], qk2s[0], t)
                    pt = ptp.tile([128, 3072], dt.bfloat16, tag="pt",
                                  name="pt")
                    # wave A1: head 0, wave-1 chunks
                    stA = psst.tile([128, 1024], dt.float32, tag="st",
                                    name="st")
                    emit_qk(qk2s[p], stA, t, W1, (0,))
                    nc.scalar.activation(
                        pt[:, 0:1024], stA[:],
                        mybir.ActivationFunctionType.Exp, scale=SCALE)
                    # wave B1: head 1, wave-1 chunks (other region)
                    stB = psst.tile([128, 1024], dt.float32, tag="st",
                                    name="st")
                    emit_qk(qk2s[p], stB, t, W1, (1,))
                    nc.scalar.activation(
                        pt[:, 1024:2048], stB[:],
                        mybir.ActivationFunctionType.Exp, scale=SCALE)
                    if t > 0:
                        ot2 = emit_pv(vtp_p, pt_prev, t - 1)
                        osbT_prev = emit_epi_head(ot2)
                    # wave 2: both heads' {c3, c5} (region = A1's, now free)
                    st2 = psst.tile([128, 1024], dt.float32, tag="st",
                                    name="st")
                    emit_qk(qk2s[p], st2, t, W2, (0, 1))
                    nc.scalar.activation(
                        pt[:, 2048:3072], st2[:],
                        mybir.ActivationFunctionType.Exp, scale=SCALE)
                    if p + 1 < NPAIR:
                        emit_prep(nats[p + 1], qk2s[p + 1], t)
                    nc.vector.tensor_tensor(
                        pt[:, 0:2048], pt[:, 0:2048], mega[:, 0:2048],
                        op=mybir.AluOpType.mult)
                    nc.gpsimd.tensor_tensor(
                        pt[:, 2048:3072], pt[:, 2048:3072],
                        mega[:, 2048:3072], op=mybir.AluOpType.mult)
                    if t > 0:
                        emit_epi_tail(osbT_prev, p, t - 1)
                    pt_prev = pt
                ot2 = emit_pv(vtp_p, pt_prev, NB - 1)
                osbT_last = emit_epi_head(ot2)
                emit_epi_tail(osbT_last, p, NB - 1)

    nc.finalize()
    return nc


_NC_CACHE = []


def _get_nc():
    if not _NC_CACHE:
        _NC_CACHE.append(build_core_kernel())
    return _NC_CACHE[0]


def make_in_maps(q, k, v):
    qr = np.ascontiguousarray(np.asarray(q, dtype=np.float32).reshape(B * H, S, D))
    kr = np.ascontiguousarray(np.asarray(k, dtype=np.float32).reshape(B * H, S, D))
    vr = np.ascontiguousarray(np.asarray(v, dtype=np.float32).reshape(B * H, S, D))
    band = np.ascontiguousarray(build_mega_mask())

    in_maps = []
    for i in range(N_CORES):
        in_maps.append({
            "q": np.ascontiguousarray(qr[BH * i:BH * (i + 1)].reshape(BH * S, D)),
            "k": np.ascontiguousarray(kr[BH * i:BH * (i + 1)].reshape(BH * S, D)),
            "v": np.ascontiguousarray(vr[BH * i:BH * (i + 1)].reshape(BH * S, D)),
            "band_mask": band,
        })
    return in_maps


def gather_out(res):
    out = np.empty((B * H, S, D), dtype=np.float32)
    for i in range(N_CORES):
        out[BH * i:BH * (i + 1)] = res.results[i]["o"].reshape(BH, S, D)
    return out.reshape(B, H, S, D)


def kernel(q, k, v):
    nc = _get_nc()
    in_maps = make_in_maps(q, k, v)
    res = bass_utils.run_bass_kernel_spmd(nc, in_maps, core_ids=list(range(N_CORES)))
    return gather_out(res)


# revision 3
# speedup vs baseline: 1.0353x; 1.0167x over previous
"""Sliding-window attention (window=256) on 8 TRN2 NeuronCores — v2.

Design (vs the fp32r baseline):
- All compute in bf16 (inputs SWDGE-cast on load): every matmul streams at
  1 cycle/row regardless of width, weight loads get FWL.
- Heads processed in PAIRS: head A's transposed Q^T/K^T live on partitions
  0-63, head B's on 64-127 (produced by fused [128,128] PE transposes of
  column-interleaved natural tiles).  The two heads' QK chunk matmuls use
  row groups (0,0)/(64,0) via base-partition-64 APs and run CONCURRENTLY
  on the PE array (contraction is only d=64).
- Transposes are regular matmuls against a bf16 identity (counts as PE-busy
  for the HAM clock gate, unlike transpose-mode), emitted interleaved with
  block compute so the PE never idles into a re-throttle window.
- Scores go to one shared [128,2048] fp32 PSUM tile (4 banks) in 2 waves:
  wave1 = chunks {c0,c2,c1,c4} of both heads, wave2 = {c3,c5} (reuses banks
  after the wave-1 exp drains).  ONE merged exp per wave on ACT
  ([128,2048] then [128,1024]) writing bf16 P^T.
- Band masking = 2 wide bf16 tensor_tensor multiplies against a precomputed
  {1,0} mega-mask (DVE 4x mode: all-SBUF, all-2-byte).
- PV is per-chunk into a [65,512] fp32 PSUM bank per head using has_written
  semantics: the c2 matmul (cols 0:384) is the only start=True (clears the
  bank's has_written bits); every other chunk accumulates where bits are set
  and overwrites where they aren't.  V carries a ones column so row 64
  accumulates the softmax denominator.
- Epilogue: O^T [65,512] is cast-copied to SBUF bf16 [80,512], transposed by
  the DMA XBAR (nc.sync dma transpose) to [128,4,80] (query q = 4p+j), then
  reciprocal + one broadcast multiply (all SBUF) and a contiguous store.
"""

import numpy as np

import concourse.bass as bass
import concourse.mybir as mybir
from concourse import bacc
from concourse.tile import TileContext
from concourse import bass_utils
from concourse.masks import make_identity

dt = mybir.dt

B, H, S, D = 4, 16, 4096, 64
W = 256
N_CORES = 8
BH = (B * H) // N_CORES      # heads per core = 8
NPAIR = BH // 2              # head pairs per core = 4
QT = 512                     # queries per block
NB = S // QT                 # blocks per head = 8
NT = S // 128                # 128-tiles per head = 32
SCALE = float(D) ** -0.5

# chunk c covers key chunk g = 4t-2+c; query window within the block:
QW = {0: (0, 128), 1: (0, 256), 2: (0, 384),
      3: (128, 512), 4: (256, 512), 5: (384, 512)}
W1 = [0, 2, 1, 4]            # wave-1 chunks (banks 0-1 per head)
W2 = [3, 5]                  # wave-2 chunks
ST1_OFF = {0: 0, 2: 128, 1: 512, 4: 768}
ST2_OFF = {3: 0, 5: 384}


def st_off(h, c):
    """Column offset of (head h, chunk c) scores within its wave's
    [128,1024] ST region (wave1 regions are per-head, wave2 is shared)."""
    if c in ST1_OFF:
        return ST1_OFF[c]
    return 512 * h + ST2_OFF[c]


def pt_off(h, c):
    """Column offset of (head h, chunk c) probabilities in the [128,3072] P^T."""
    if c in ST1_OFF:
        return 1024 * h + ST1_OFF[c]
    return 2048 + 512 * h + ST2_OFF[c]


def build_mega_mask():
    """{1,0} float32 [128, 3072] band mask laid out to match pt_off."""
    m = np.zeros((128, 3072), dtype=np.float32)
    kl = np.arange(128)[:, None]
    for h in (0, 1):
        for c in range(6):
            q0, q1 = QW[c]
            qi = np.arange(q0, q1)[None, :]
            key = 128 * (c - 2) + kl
            valid = (key >= qi - 256) & (key <= qi)
            off = pt_off(h, c)
            m[:, off:off + (q1 - q0)] = valid.astype(np.float32)
    return m


def build_core_kernel(n_bh=BH):
    nc = bacc.Bacc("TRN2", target_bir_lowering=False)
    qd = nc.dram_tensor("q", [n_bh * S, D], dt.float32, kind="ExternalInput")
    kd = nc.dram_tensor("k", [n_bh * S, D], dt.float32, kind="ExternalInput")
    vd = nc.dram_tensor("v", [n_bh * S, D], dt.float32, kind="ExternalInput")
    md = nc.dram_tensor("band_mask", [128, 3072], dt.float32,
                        kind="ExternalInput")
    od = nc.dram_tensor("o", [n_bh * S, D], dt.float32, kind="ExternalOutput")

    with TileContext(nc) as tc:
        with (
            tc.tile_pool(name="const", bufs=1) as constp,
            tc.tile_pool(name="bigio", bufs=2) as bigio,
            tc.tile_pool(name="qkt", bufs=2) as qktp,
            tc.tile_pool(name="ptp", bufs=3) as ptp,
            tc.tile_pool(name="work", bufs=2) as work,
            tc.tile_pool(name="psst", bufs=3, space="PSUM") as psst,
            tc.tile_pool(name="psot", bufs=1, space="PSUM") as psot,
        ):
            ident = constp.tile([128, 128], dt.float32)
            make_identity(nc, ident)
            identb = constp.tile([128, 128], dt.bfloat16)
            nc.vector.tensor_copy(identb[:], ident[:])
            mega = constp.tile([128, 3072], dt.bfloat16)
            nc.gpsimd.dma_start(mega[:], md[:])  # cast fp32 -> bf16

            def emit_loads(p):
                """SWDGE cast-loads for pair p: q/k interleaved by head within
                each 128-tile's columns, v per (chunk, head)."""
                base = 2 * p * S
                qnat = bigio.tile([128, NT * 128], dt.bfloat16, tag="qnat",
                                  name=f"qnat{p}")
                knat = bigio.tile([128, NT * 128], dt.bfloat16, tag="knat",
                                  name=f"knat{p}")
                for dn, nat in ((qd, qnat), (kd, knat)):
                    dst4 = nat[:].rearrange("p (t h d) -> p t h d", h=2, d=64)
                    for h in (0, 1):
                        src = dn[base + h * S:base + (h + 1) * S, :].rearrange(
                            "(t p) d -> p t d", p=128)
                        nc.gpsimd.dma_start(dst4[:, :, h, :], src)
                vtp = bigio.tile([128, NT * 2 * 65], dt.bfloat16, tag="vtp",
                                 name=f"vtp{p}")
                vt4 = vtp[:].rearrange("p (g h e) -> p g h e", h=2, e=65)
                for h in (0, 1):
                    vsrc = vd[base + h * S:base + (h + 1) * S, :].rearrange(
                        "(g p) d -> p g d", p=128)
                    nc.gpsimd.dma_start(vt4[:, :, h, 0:D], vsrc)
                nc.vector.memset(vt4[:, :, :, D], 1.0)
                return qnat, knat, vtp

            def alloc_qkt(p):
                qt2 = qktp.tile([128, S], dt.bfloat16, tag="qt2",
                                name=f"qt2_{p}")
                kt2 = qktp.tile([128, S], dt.bfloat16, tag="kt2",
                                name=f"kt2_{p}")
                return qt2, kt2

            def emit_prep(nats, qk2, t):
                """Transpose tiles 4t..4t+3 of q and k for the NEXT pair.
                Transpose-mode with bf16 PSUM output so the copy-out runs at
                the DVE 2-byte rate; PSUM comes from the st pool rotation
                (2KB per tile = same bank footprint as a score tile)."""
                qnat, knat, _ = nats
                qt2, kt2 = qk2
                for nat, dst in ((qnat, qt2), (knat, kt2)):
                    tr = psst.tile([128, 2048], dt.bfloat16, tag="st",
                                   name="tr")
                    for u in range(4):
                        i = 4 * t + u
                        nc.tensor.transpose(
                            tr[:, 128 * u:128 * (u + 1)],
                            nat[:, 128 * i:128 * (i + 1)], identb[:])
                    nc.vector.tensor_copy(dst[:, 512 * t:512 * (t + 1)],
                                          tr[:, 0:512])

            def emit_qk(qk2, st, t, chunks, heads):
                qt2, kt2 = qk2
                for c in chunks:
                    g = 4 * t - 2 + c
                    if g < 0:
                        continue
                    q0, q1 = QW[c]
                    for h in heads:
                        b = 64 * h
                        nc.tensor.matmul(
                            st[:, st_off(h, c):st_off(h, c) + (q1 - q0)],
                            kt2[b:b + 64, 128 * g:128 * (g + 1)],
                            qt2[b:b + 64, QT * t + q0:QT * t + q1],
                            start=True, stop=True)

            def emit_pv(vtp, pt, t):
                vt4 = vtp[:].rearrange("p (g h e) -> p g h e", h=2, e=65)
                chunks = [c for c in range(6) if 4 * t - 2 + c >= 0]
                order = [2] + [c for c in chunks if c != 2]
                ot2 = psot.tile([65, 2 * QT], dt.float32, tag="ot2",
                                name="ot2")
                for idx, c in enumerate(order):
                    g = 4 * t - 2 + c
                    q0, q1 = QW[c]
                    for h in (0, 1):
                        nc.tensor.matmul(
                            ot2[:, QT * h + q0:QT * h + q1],
                            vt4[:, g, h, :],
                            pt[:, pt_off(h, c):pt_off(h, c) + (q1 - q0)],
                            start=(idx == 0), stop=(idx == len(order) - 1),
                            skip_group_check=True)
                return ot2

            def emit_epi_head(ot2):
                """osb cast + dma transpose for both heads (merged)."""
                osb = work.tile([80, 2 * QT], dt.bfloat16, tag="osb",
                                name="osb")
                nc.vector.tensor_copy(osb[0:65, :], ot2[:])
                osbT = work.tile([128, 8 * 80], dt.bfloat16,
                                 tag="osbT", name="osbT")
                o3 = osbT[:].rearrange("p (j e) -> p j e", e=80)
                nc.sync.dma_start(o3, osb[:], transpose=True)
                return osbT

            def emit_epi_tail(osbT, p, t):
                o3 = osbT[:].rearrange("p (j e) -> p j e", e=80)
                rc = work.tile([128, 8], dt.float32, tag="rc", name="rc")
                nc.vector.reciprocal(rc[:], o3[:, :, D])
                outsb = work.tile([128, 512], dt.float32, tag="outsb",
                                  name="outsb")
                u3 = outsb[:].rearrange("p (j e) -> p j e", e=64)
                rcb = rc[:].rearrange("p (j o) -> p j o", o=1)
                nc.vector.tensor_tensor(
                    u3, o3[:, :, 0:D], rcb.broadcast_to([128, 8, 64]),
                    op=mybir.AluOpType.mult)
                for h in (0, 1):
                    base = (2 * p + h) * S
                    # dma transpose is j-major: query q = 128j + p
                    dst = od[base + QT * t:base + QT * (t + 1), :].rearrange(
                        "(j p) d -> p j d", p=128)
                    nc.sync.dma_start(dst, u3[:, 4 * h:4 * h + 4, :])

            # ---- prologue: loads for pairs 0/1, transposes for pair 0
            nats = [emit_loads(0), emit_loads(1)]
            qk2s = [alloc_qkt(0)]
            for t in range(NB):
                emit_prep(nats[0], qk2s[0], t)

            for p in range(NPAIR):
                if p + 2 < NPAIR:
                    nats.append(emit_loads(p + 2))
                if p + 1 < NPAIR:
                    qk2s.append(alloc_qkt(p + 1))
                vtp_p = nats[p][2]
                for t in range(NB):
                    pt = ptp.tile([128, 3072], dt.bfloat16, tag="pt",
                                  name="pt")
                    # wave A1: head 0, wave-1 chunks
                    stA = psst.tile([128, 1024], dt.float32, tag="st",
                                    name="st")
                    emit_qk(qk2s[p], stA, t, W1, (0,))
                    nc.scalar.activation(
                        pt[:, 0:1024], stA[:],
                        mybir.ActivationFunctionType.Exp, scale=SCALE)
                    # wave B1: head 1, wave-1 chunks (other region)
                    stB = psst.tile([128, 1024], dt.float32, tag="st",
                                    name="st")
                    emit_qk(qk2s[p], stB, t, W1, (1,))
                    nc.scalar.activation(
                        pt[:, 1024:2048], stB[:],
                        mybir.ActivationFunctionType.Exp, scale=SCALE)
                    if t > 0:
                        ot2 = emit_pv(vtp_p, pt_prev, t - 1)
                        osbT_prev = emit_epi_head(ot2)
                    # wave 2: both heads' {c3, c5} (region = A1's, now free)
                    st2 = psst.tile([128, 1024], dt.float32, tag="st",
                                    name="st")
                    emit_qk(qk2s[p], st2, t, W2, (0, 1))
                    nc.scalar.activation(
                        pt[:, 2048:3072], st2[:],
                        mybir.ActivationFunctionType.Exp, scale=SCALE)
                    if p + 1 < NPAIR:
                        emit_prep(nats[p + 1], qk2s[p + 1], t)
                    nc.vector.tensor_tensor(
                        pt[:, 0:2048], pt[:, 0:2048], mega[:, 0:2048],
                        op=mybir.AluOpType.mult)
                    nc.gpsimd.tensor_tensor(
                        pt[:, 2048:3072], pt[:, 2048:3072],
                        mega[:, 2048:3072], op=mybir.AluOpType.mult)
                    if t > 0:
                        emit_epi_tail(osbT_prev, p, t - 1)
                    pt_prev = pt
                ot2 = emit_pv(vtp_p, pt_prev, NB - 1)
                osbT_last = emit_epi_head(ot2)
                emit_epi_tail(osbT_last, p, NB - 1)

    nc.finalize()
    return nc


_NC_CACHE = []


def _get_nc():
    if not _NC_CACHE:
        _NC_CACHE.append(build_core_kernel())
    return _NC_CACHE[0]


def make_in_maps(q, k, v):
    qr = np.ascontiguousarray(np.asarray(q, dtype=np.float32).reshape(B * H, S, D))
    kr = np.ascontiguousarray(np.asarray(k, dtype=np.float32).reshape(B * H, S, D))
    vr = np.ascontiguousarray(np.asarray(v, dtype=np.float32).reshape(B * H, S, D))
    band = np.ascontiguousarray(build_mega_mask())

    in_maps = []
    for i in range(N_CORES):
        in_maps.append({
            "q": np.ascontiguousarray(qr[BH * i:BH * (i + 1)].reshape(BH * S, D)),
            "k": np.ascontiguousarray(kr[BH * i:BH * (i + 1)].reshape(BH * S, D)),
            "v": np.ascontiguousarray(vr[BH * i:BH * (i + 1)].reshape(BH * S, D)),
            "band_mask": band,
        })
    return in_maps


def gather_out(res):
    out = np.empty((B * H, S, D), dtype=np.float32)
    for i in range(N_CORES):
        out[BH * i:BH * (i + 1)] = res.results[i]["o"].reshape(BH, S, D)
    return out.reshape(B, H, S, D)


def kernel(q, k, v):
    nc = _get_nc()
    in_maps = make_in_maps(q, k, v)
    res = bass_utils.run_bass_kernel_spmd(nc, in_maps, core_ids=list(range(N_CORES)))
    return gather_out(res)


# revision 4
# speedup vs baseline: 1.0895x; 1.0523x over previous
"""Sliding-window attention (window=256) on 8 TRN2 NeuronCores — v2.

Design (vs the fp32r baseline):
- All compute in bf16 (inputs SWDGE-cast on load): every matmul streams at
  1 cycle/row regardless of width, weight loads get FWL.
- Heads processed in PAIRS: head A's transposed Q^T/K^T live on partitions
  0-63, head B's on 64-127 (produced by fused [128,128] PE transposes of
  column-interleaved natural tiles).  The two heads' QK chunk matmuls use
  row groups (0,0)/(64,0) via base-partition-64 APs and run CONCURRENTLY
  on the PE array (contraction is only d=64).
- Transposes are regular matmuls against a bf16 identity (counts as PE-busy
  for the HAM clock gate, unlike transpose-mode), emitted interleaved with
  block compute so the PE never idles into a re-throttle window.
- Scores go to one shared [128,2048] fp32 PSUM tile (4 banks) in 2 waves:
  wave1 = chunks {c0,c2,c1,c4} of both heads, wave2 = {c3,c5} (reuses banks
  after the wave-1 exp drains).  ONE merged exp per wave on ACT
  ([128,2048] then [128,1024]) writing bf16 P^T.
- Band masking = 2 wide bf16 tensor_tensor multiplies against a precomputed
  {1,0} mega-mask (DVE 4x mode: all-SBUF, all-2-byte).
- PV is per-chunk into a [65,512] fp32 PSUM bank per head using has_written
  semantics: the c2 matmul (cols 0:384) is the only start=True (clears the
  bank's has_written bits); every other chunk accumulates where bits are set
  and overwrites where they aren't.  V carries a ones column so row 64
  accumulates the softmax denominator.
- Epilogue: O^T [65,512] is cast-copied to SBUF bf16 [80,512], transposed by
  the DMA XBAR (nc.sync dma transpose) to [128,4,80] (query q = 4p+j), then
  reciprocal + one broadcast multiply (all SBUF) and a contiguous store.
"""

import numpy as np

import concourse.bass as bass
import concourse.mybir as mybir
from concourse import bacc
from concourse.tile import TileContext
from concourse import bass_utils
from concourse.masks import make_identity

dt = mybir.dt

B, H, S, D = 4, 16, 4096, 64
W = 256
N_CORES = 8
BH = (B * H) // N_CORES      # heads per core = 8
NPAIR = BH // 2              # head pairs per core = 4
QT = 512                     # queries per block
NB = S // QT                 # blocks per head = 8
NT = S // 128                # 128-tiles per head = 32
SCALE = float(D) ** -0.5

# chunk c covers key chunk g = 4t-2+c; query window within the block:
QW = {0: (0, 128), 1: (0, 256), 2: (0, 384),
      3: (128, 512), 4: (256, 512), 5: (384, 512)}
W1 = [0, 2, 1, 4]            # wave-1 chunks (banks 0-1 per head)
W2 = [3, 5]                  # wave-2 chunks
ST1_OFF = {0: 0, 2: 128, 1: 512, 4: 768}
ST2_OFF = {3: 0, 5: 384}


def st_off(h, c):
    """Column offset of (head h, chunk c) scores within its wave's
    [128,1024] ST region (wave1 regions are per-head, wave2 is shared)."""
    if c in ST1_OFF:
        return ST1_OFF[c]
    return 512 * h + ST2_OFF[c]


def pt_off(h, c):
    """Column offset of (head h, chunk c) probabilities in the [128,3072] P^T."""
    if c in ST1_OFF:
        return 1024 * h + ST1_OFF[c]
    return 2048 + 512 * h + ST2_OFF[c]


def build_mega_mask():
    """{1,0} float32 [128, 3072] band mask laid out to match pt_off."""
    m = np.zeros((128, 3072), dtype=np.float32)
    kl = np.arange(128)[:, None]
    for h in (0, 1):
        for c in range(6):
            q0, q1 = QW[c]
            qi = np.arange(q0, q1)[None, :]
            key = 128 * (c - 2) + kl
            valid = (key >= qi - 256) & (key <= qi)
            off = pt_off(h, c)
            m[:, off:off + (q1 - q0)] = valid.astype(np.float32)
    return m


def build_core_kernel(n_bh=BH):
    nc = bacc.Bacc("TRN2", target_bir_lowering=False)
    qd = nc.dram_tensor("q", [n_bh * S, D], dt.float32, kind="ExternalInput")
    kd = nc.dram_tensor("k", [n_bh * S, D], dt.float32, kind="ExternalInput")
    vd = nc.dram_tensor("v", [n_bh * S, D], dt.float32, kind="ExternalInput")
    md = nc.dram_tensor("band_mask", [128, 3072], dt.float32,
                        kind="ExternalInput")
    od = nc.dram_tensor("o", [n_bh * S, D], dt.float32, kind="ExternalOutput")

    with TileContext(nc) as tc:
        with (
            tc.tile_pool(name="const", bufs=1) as constp,
            tc.tile_pool(name="bigio", bufs=2) as bigio,
            tc.tile_pool(name="qkt", bufs=2) as qktp,
            tc.tile_pool(name="ptp", bufs=3) as ptp,
            tc.tile_pool(name="work", bufs=2) as work,
            tc.tile_pool(name="psst", bufs=3, space="PSUM") as psst,
            tc.tile_pool(name="psot", bufs=1, space="PSUM") as psot,
        ):
            ident = constp.tile([128, 128], dt.float32)
            make_identity(nc, ident)
            identb = constp.tile([128, 128], dt.bfloat16)
            nc.vector.tensor_copy(identb[:], ident[:])
            mega = constp.tile([128, 3072], dt.bfloat16)
            nc.gpsimd.dma_start(mega[:], md[:])  # cast fp32 -> bf16

            def emit_load_nat(p, dn, tag):
                """SWDGE cast-load of q or k for pair p, head-interleaved
                within each 128-tile's columns."""
                base = 2 * p * S
                nat = bigio.tile([128, NT * 128], dt.bfloat16, tag=tag,
                                 name=f"{tag}{p}")
                dst4 = nat[:].rearrange("p (t h d) -> p t h d", h=2, d=64)
                for h in (0, 1):
                    srcap = dn[base + h * S:base + (h + 1) * S, :].rearrange(
                        "(t p) d -> p t d", p=128)
                    nc.gpsimd.dma_start(dst4[:, :, h, :], srcap)
                return nat

            def emit_load_v(p):
                base = 2 * p * S
                vtp = bigio.tile([128, NT * 2 * 65], dt.bfloat16, tag="vtp",
                                 name=f"vtp{p}")
                vt4 = vtp[:].rearrange("p (g h e) -> p g h e", h=2, e=65)
                for h in (0, 1):
                    vsrc = vd[base + h * S:base + (h + 1) * S, :].rearrange(
                        "(g p) d -> p g d", p=128)
                    nc.gpsimd.dma_start(vt4[:, :, h, 0:D], vsrc)
                nc.vector.memset(vt4[:, :, :, D], 1.0)
                return vtp

            def alloc_qkt(p):
                qt2 = qktp.tile([128, S], dt.bfloat16, tag="qt2",
                                name=f"qt2_{p}")
                kt2 = qktp.tile([128, S], dt.bfloat16, tag="kt2",
                                name=f"kt2_{p}")
                return qt2, kt2

            def emit_prep(nats, qk2, t):
                """Transpose tiles 4t..4t+3 of q and k for the NEXT pair.
                Transpose-mode with bf16 PSUM output so the copy-out runs at
                the DVE 2-byte rate; PSUM comes from the st pool rotation
                (2KB per tile = same bank footprint as a score tile)."""
                qnat, knat, _ = nats
                qt2, kt2 = qk2
                for nat, dst in ((qnat, qt2), (knat, kt2)):
                    tr = psst.tile([128, 2048], dt.bfloat16, tag="st",
                                   name="tr")
                    for u in range(4):
                        i = 4 * t + u
                        nc.tensor.transpose(
                            tr[:, 128 * u:128 * (u + 1)],
                            nat[:, 128 * i:128 * (i + 1)], identb[:])
                    nc.vector.tensor_copy(dst[:, 512 * t:512 * (t + 1)],
                                          tr[:, 0:512])

            def emit_qk(qk2, st, t, chunks, heads):
                qt2, kt2 = qk2
                for c in chunks:
                    g = 4 * t - 2 + c
                    if g < 0:
                        continue
                    q0, q1 = QW[c]
                    for h in heads:
                        b = 64 * h
                        nc.tensor.matmul(
                            st[:, st_off(h, c):st_off(h, c) + (q1 - q0)],
                            kt2[b:b + 64, 128 * g:128 * (g + 1)],
                            qt2[b:b + 64, QT * t + q0:QT * t + q1],
                            start=True, stop=True)

            def emit_pv(vtp, pt, t):
                vt4 = vtp[:].rearrange("p (g h e) -> p g h e", h=2, e=65)
                chunks = [c for c in range(6) if 4 * t - 2 + c >= 0]
                order = [2] + [c for c in chunks if c != 2]
                ot2 = psot.tile([65, 2 * QT], dt.float32, tag="ot2",
                                name="ot2")
                for idx, c in enumerate(order):
                    g = 4 * t - 2 + c
                    q0, q1 = QW[c]
                    for h in (0, 1):
                        nc.tensor.matmul(
                            ot2[:, QT * h + q0:QT * h + q1],
                            vt4[:, g, h, :],
                            pt[:, pt_off(h, c):pt_off(h, c) + (q1 - q0)],
                            start=(idx == 0), stop=(idx == len(order) - 1),
                            skip_group_check=True)
                return ot2

            def emit_epi_head(ot2):
                """osb cast + dma transpose for both heads (merged)."""
                osb = work.tile([80, 2 * QT], dt.bfloat16, tag="osb",
                                name="osb")
                nc.vector.tensor_copy(osb[0:65, :], ot2[:])
                osbT = work.tile([128, 8 * 80], dt.bfloat16,
                                 tag="osbT", name="osbT")
                o3 = osbT[:].rearrange("p (j e) -> p j e", e=80)
                nc.sync.dma_start(o3, osb[:], transpose=True)
                return osbT

            def emit_epi_tail(osbT, p, t):
                o3 = osbT[:].rearrange("p (j e) -> p j e", e=80)
                rc = work.tile([128, 8], dt.float32, tag="rc", name="rc")
                nc.vector.reciprocal(rc[:], o3[:, :, D])
                outsb = work.tile([128, 512], dt.float32, tag="outsb",
                                  name="outsb")
                u3 = outsb[:].rearrange("p (j e) -> p j e", e=64)
                rcb = rc[:].rearrange("p (j o) -> p j o", o=1)
                nc.vector.tensor_tensor(
                    u3, o3[:, :, 0:D], rcb.broadcast_to([128, 8, 64]),
                    op=mybir.AluOpType.mult)
                for h in (0, 1):
                    base = (2 * p + h) * S
                    # dma transpose is j-major: query q = 128j + p
                    dst = od[base + QT * t:base + QT * (t + 1), :].rearrange(
                        "(j p) d -> p j d", p=128)
                    nc.sync.dma_start(dst, u3[:, 4 * h:4 * h + 4, :])

            # ---- prologue: q/k for pairs 0/1 and v for pair 0 up front;
            # later pairs' loads are spread across earlier pairs' blocks to
            # keep the SDMA queue shallow for epilogue DMAs
            nats = [[emit_load_nat(0, qd, "qnat"), emit_load_nat(0, kd, "knat"),
                     emit_load_v(0)],
                    [emit_load_nat(1, qd, "qnat"), emit_load_nat(1, kd, "knat"),
                     None]]
            qk2s = [alloc_qkt(0)]
            for t in range(NB):
                emit_prep(nats[0], qk2s[0], t)

            for p in range(NPAIR):
                if p + 2 < NPAIR:
                    nats.append([None, None, None])
                if p + 1 < NPAIR:
                    qk2s.append(alloc_qkt(p + 1))
                vtp_p = nats[p][2]
                for t in range(NB):
                    if t == 0 and p + 1 < NPAIR and nats[p + 1][2] is None:
                        nats[p + 1][2] = emit_load_v(p + 1)
                    if p + 2 < NPAIR:
                        if t == 2:
                            nats[p + 2][0] = emit_load_nat(p + 2, qd, "qnat")
                        elif t == 4:
                            nats[p + 2][1] = emit_load_nat(p + 2, kd, "knat")
                    pt = ptp.tile([128, 3072], dt.bfloat16, tag="pt",
                                  name="pt")
                    # wave A1: head 0, wave-1 chunks
                    stA = psst.tile([128, 1024], dt.float32, tag="st",
                                    name="st")
                    emit_qk(qk2s[p], stA, t, W1, (0,))
                    nc.scalar.activation(
                        pt[:, 0:1024], stA[:],
                        mybir.ActivationFunctionType.Exp, scale=SCALE)
                    # wave B1: head 1, wave-1 chunks (other region)
                    stB = psst.tile([128, 1024], dt.float32, tag="st",
                                    name="st")
                    emit_qk(qk2s[p], stB, t, W1, (1,))
                    nc.scalar.activation(
                        pt[:, 1024:2048], stB[:],
                        mybir.ActivationFunctionType.Exp, scale=SCALE)
                    if t > 0:
                        ot2 = emit_pv(vtp_p, pt_prev, t - 1)
                        osbT_prev = emit_epi_head(ot2)
                    # wave 2: both heads' {c3, c5} (region = A1's, now free)
                    st2 = psst.tile([128, 1024], dt.float32, tag="st",
                                    name="st")
                    emit_qk(qk2s[p], st2, t, W2, (0, 1))
                    nc.scalar.activation(
                        pt[:, 2048:3072], st2[:],
                        mybir.ActivationFunctionType.Exp, scale=SCALE)
                    if p + 1 < NPAIR:
                        emit_prep(nats[p + 1], qk2s[p + 1], t)
                    nc.vector.tensor_tensor(
                        pt[:, 0:2048], pt[:, 0:2048], mega[:, 0:2048],
                        op=mybir.AluOpType.mult)
                    nc.vector.tensor_tensor(
                        pt[:, 2048:2816], pt[:, 2048:2816],
                        mega[:, 2048:2816], op=mybir.AluOpType.mult)
                    nc.gpsimd.tensor_tensor(
                        pt[:, 2816:3072], pt[:, 2816:3072],
                        mega[:, 2816:3072], op=mybir.AluOpType.mult)
                    if t > 0:
                        emit_epi_tail(osbT_prev, p, t - 1)
                    pt_prev = pt
                ot2 = emit_pv(vtp_p, pt_prev, NB - 1)
                osbT_last = emit_epi_head(ot2)
                emit_epi_tail(osbT_last, p, NB - 1)

    nc.finalize()
    return nc


_NC_CACHE = []


def _get_nc():
    if not _NC_CACHE:
        _NC_CACHE.append(build_core_kernel())
    return _NC_CACHE[0]


def make_in_maps(q, k, v):
    qr = np.ascontiguousarray(np.asarray(q, dtype=np.float32).reshape(B * H, S, D))
    kr = np.ascontiguousarray(np.asarray(k, dtype=np.float32).reshape(B * H, S, D))
    vr = np.ascontiguousarray(np.asarray(v, dtype=np.float32).reshape(B * H, S, D))
    band = np.ascontiguousarray(build_mega_mask())

    in_maps = []
    for i in range(N_CORES):
        in_maps.append({
            "q": np.ascontiguousarray(qr[BH * i:BH * (i + 1)].reshape(BH * S, D)),
            "k": np.ascontiguousarray(kr[BH * i:BH * (i + 1)].reshape(BH * S, D)),
            "v": np.ascontiguousarray(vr[BH * i:BH * (i + 1)].reshape(BH * S, D)),
            "band_mask": band,
        })
    return in_maps


def gather_out(res):
    out = np.empty((B * H, S, D), dtype=np.float32)
    for i in range(N_CORES):
        out[BH * i:BH * (i + 1)] = res.results[i]["o"].reshape(BH, S, D)
    return out.reshape(B, H, S, D)


def kernel(q, k, v):
    nc = _get_nc()
    in_maps = make_in_maps(q, k, v)
    res = bass_utils.run_bass_kernel_spmd(nc, in_maps, core_ids=list(range(N_CORES)))
    return gather_out(res)
